# revision 1
# baseline (speedup 1.0000x reference)
"""Per-row cosine-similarity loss (0.5 * cos(x1_row, x2_row)) on 8 TRN2 cores.

Pure data parallel: the batch dim (B=16384) is split into 8 shards of 2048
rows; each core computes its shard independently, no communication.

Per-core kernel (shard = [2048, 4096] f32 per tensor):
  - rows are tiled as row = p*16 + n  (p = SBUF partition, n = tile index),
    so each [128, 4096] tile is one ACT/DVE instruction and the final
    per-row result lands in a [128, 16] tile that stores with one DMA.
  - ACT (scalar engine): Square activation with accum_out -> per-row sum of
    squares for x1 and x2 (fused square+reduce, one pass per tensor).
  - DVE (vector engine): scalar_tensor_tensor(mult, mult, accum_out) ->
    per-row dot product (fused multiply+reduce, one pass).
  - Final [128, 16] math: cos = dot / (2*sqrt(sx)*sqrt(sy)) using
    sqrt(4*sx) = 2*sqrt(sx) to fold in the 0.5 factor.

The kernel is HBM-bound: 64 MiB input per core @ ~358 GB/s => ~187 us floor.
"""

import numpy as np

import concourse.bacc as bacc
import concourse.bass as bass
import concourse.tile as tile
from concourse import mybir
from concourse.bass_utils import run_bass_kernel_spmd

B, D = 16384, 4096
N_CORES = 8
B_SHARD = B // N_CORES  # 2048
P = 128
N_TILES = B_SHARD // P  # 16

_NC_CACHE = None
# kernel layout used by kernel(); host gather must match build_kernel()
SEQ_LAYOUT = False


def build_kernel(
    repeat: int = 1,
    bufs: int = 4,
    split_rings: bool = False,
    dma_merge: int = 1,
    inc_finalize: bool = False,
    seq_layout: bool = False,
    split_tail: bool = False,
) -> bass.Bass:
    # Bacc (not plain Bass): its compile() pass legalizes instructions that
    # carry multiple sync waits, which walrus rejects from raw Bass output.
    # `repeat` re-runs the whole tile loop (same data, same output) and is
    # only used for marginal-timing benchmarks; keep 1 for real use.
    nc = bacc.Bacc("TRN2", target_bir_lowering=False)
    f32 = mybir.dt.float32

    x1 = nc.dram_tensor("x1", [B_SHARD, D], f32, kind="ExternalInput")
    x2 = nc.dram_tensor("x2", [B_SHARD, D], f32, kind="ExternalInput")

    if seq_layout:
        # row = n*128 + p: every [128, D] tile is one fully-contiguous 2 MiB
        # block and the 16 tiles stream HBM perfectly sequentially. The
        # per-row results then land in out[p, n] = row n*128+p, which the
        # host unscrambles with a free transpose (see kernel()).
        out = nc.dram_tensor("out", [P, N_TILES], f32, kind="ExternalOutput")
        x1r = x1.rearrange("(n p) d -> p n d", p=P)  # [128, 16, D]
        x2r = x2.rearrange("(n p) d -> p n d", p=P)
        outr = out[:, :]  # [128, 16]
    else:
        # row = p*N_TILES + n: tile n is [128, D] with partition stride
        # N_TILES*D (16 KiB contiguous per partition, 256 KiB stride).
        out = nc.dram_tensor("out", [B_SHARD], f32, kind="ExternalOutput")
        x1r = x1.rearrange("(p n) d -> p n d", p=P)  # [128, 16, D]
        x2r = x2.rearrange("(p n) d -> p n d", p=P)
        outr = out.rearrange("(p n) -> p n", p=P)  # [128, 16]
    # With dma_merge=m, one DMA loads m consecutive n-columns ([128, m, D]);
    # compute still runs per n-column (accum_out is one scalar per row).

    with tile.TileContext(nc) as tc:
        with (
            tc.tile_pool(name="x1p", bufs=bufs) as x1p,
            tc.tile_pool(name="x2p", bufs=bufs) as x2p,
            tc.tile_pool(name="junk", bufs=1) as junkp,
            tc.tile_pool(name="stats", bufs=1) as statsp,
        ):
            sx = statsp.tile([P, N_TILES], f32)
            sy = statsp.tile([P, N_TILES], f32)
            dot = statsp.tile([P, N_TILES], f32)
            # Mandatory full-size outputs of the fused reduce ops; never read.
            junk_a = junkp.tile([P, D], f32)
            junk_v = junkp.tile([P, D], f32)

            m = dma_merge
            assert N_TILES % m == 0
            if split_tail:
                assert m == 1 and not inc_finalize
                # partial accums for the split halves of the last tile
                part = statsp.tile([P, 4], f32, name="part")

            ssx = statsp.tile([P, N_TILES], f32, name="ssx")
            ssy = statsp.tile([P, N_TILES], f32, name="ssy")
            den = statsp.tile([P, N_TILES], f32, name="den")
            rec = statsp.tile([P, N_TILES], f32, name="rec")
            res = statsp.tile([P, N_TILES], f32, name="res")

            def finalize_col(n):
                # per-column finalize while later tiles still stream in;
                # keeps only the last column's short chain in the tail
                c = slice(n, n + 1)
                nc.scalar.activation(
                    out=ssx[:, c], in_=sx[:, c],
                    func=mybir.ActivationFunctionType.Sqrt, scale=4.0,
                )
                nc.scalar.activation(
                    out=ssy[:, c], in_=sy[:, c],
                    func=mybir.ActivationFunctionType.Sqrt,
                )
                nc.vector.tensor_mul(den[:, c], ssx[:, c], ssy[:, c])
                nc.vector.reciprocal(rec[:, c], den[:, c])
                nc.vector.tensor_mul(res[:, c], dot[:, c], rec[:, c])
                # issue from the ACT HW-DGE ring: the SP ring is the dense
                # input-DMA critical path and must not carry the tiny stores
                nc.scalar.dma_start(out=outr[:, c], in_=res[:, c])

            def split_last_tile():
                # Load/compute the last tile in two half-width pieces so the
                # tail after the final byte lands is a half-width dot instead
                # of a full one (~2 us shorter kernel tail). Half sums go to
                # `part` and are combined with one tensor_add per stat.
                n = N_TILES - 1
                H = D // 2
                t1 = x1p.tile([P, D], f32, name="t1")
                t2 = x2p.tile([P, D], f32, name="t2")
                for h in (0, 1):
                    cs = slice(h * H, (h + 1) * H)
                    nc.sync.dma_start(out=t1[:, cs], in_=x1r[:, n, cs])
                    nc.sync.dma_start(out=t2[:, cs], in_=x2r[:, n, cs])
                    nc.scalar.activation(
                        out=junk_a[:, cs],
                        in_=t1[:, cs],
                        func=mybir.ActivationFunctionType.Square,
                        accum_out=(sx[:, n : n + 1] if h == 0 else part[:, 0:1]),
                    )
                    nc.scalar.activation(
                        out=junk_a[:, cs],
                        in_=t2[:, cs],
                        func=mybir.ActivationFunctionType.Square,
                        accum_out=(sy[:, n : n + 1] if h == 0 else part[:, 1:2]),
                    )
                    nc.vector.scalar_tensor_tensor(
                        out=junk_v[:, cs],
                        in0=t1[:, cs],
                        scalar=1.0,
                        in1=t2[:, cs],
                        op0=mybir.AluOpType.mult,
                        op1=mybir.AluOpType.mult,
                        accum_out=(dot[:, n : n + 1] if h == 0 else part[:, 2:3]),
                    )
                nc.vector.tensor_add(sx[:, n : n + 1], sx[:, n : n + 1], part[:, 0:1])
                nc.vector.tensor_add(sy[:, n : n + 1], sy[:, n : n + 1], part[:, 1:2])
                nc.vector.tensor_add(dot[:, n : n + 1], dot[:, n : n + 1], part[:, 2:3])

            def tile_body():
                n_groups = N_TILES // m
                if split_tail:
                    n_groups -= 1
                for g in range(n_groups):
                    n0 = g * m
                    t1 = x1p.tile([P, m, D], f32, name="t1")
                    t2 = x2p.tile([P, m, D], f32, name="t2")
                    nc.sync.dma_start(out=t1, in_=x1r[:, n0 : n0 + m, :])
                    # optionally issue x2 loads from the ACT sequencer so the
                    # two input streams use both HW-DGE rings
                    x2_eng = nc.scalar if split_rings else nc.sync
                    x2_eng.dma_start(out=t2, in_=x2r[:, n0 : n0 + m, :])
                    for j in range(m):
                        n = n0 + j
                        nc.scalar.activation(
                            out=junk_a,
                            in_=t1[:, j, :],
                            func=mybir.ActivationFunctionType.Square,
                            accum_out=sx[:, n : n + 1],
                        )
                        nc.scalar.activation(
                            out=junk_a,
                            in_=t2[:, j, :],
                            func=mybir.ActivationFunctionType.Square,
                            accum_out=sy[:, n : n + 1],
                        )
                        # Fused (t1*1.0)*t2 with accum_out = per-row sum -> dot.
                        # (tensor_tensor_reduce compiles but faults on HW; this
                        # TensorScalarPtr form is the supported fused mul+reduce.)
                        nc.vector.scalar_tensor_tensor(
                            out=junk_v,
                            in0=t1[:, j, :],
                            scalar=1.0,
                            in1=t2[:, j, :],
                            op0=mybir.AluOpType.mult,
                            op1=mybir.AluOpType.mult,
                            accum_out=dot[:, n : n + 1],
                        )
                        if inc_finalize:
                            finalize_col(n)
                if split_tail:
                    split_last_tile()

            if repeat == 1:
                tile_body()
            else:
                with tc.For_i(0, repeat, 1):
                    tile_body()

            if not inc_finalize:
                # cos/2 = dot / (2*sqrt(sx)*sqrt(sy));  sqrt(4*sx) = 2*sqrt(sx)
                nc.scalar.activation(
                    out=ssx, in_=sx, func=mybir.ActivationFunctionType.Sqrt,
                    scale=4.0,
                )
                nc.scalar.activation(
                    out=ssy, in_=sy, func=mybir.ActivationFunctionType.Sqrt
                )
                nc.vector.tensor_mul(den, ssx, ssy)
                nc.vector.reciprocal(rec, den)
                nc.vector.tensor_mul(res, dot, rec)
                nc.sync.dma_start(out=outr, in_=res)

    nc.compile()
    return nc


def kernel(x1: np.ndarray, x2: np.ndarray, **_kw) -> np.ndarray:
    global _NC_CACHE
    x1 = np.ascontiguousarray(np.asarray(x1, dtype=np.float32))
    x2 = np.ascontiguousarray(np.asarray(x2, dtype=np.float32))
    assert x1.shape == (B, D) and x2.shape == (B, D)

    in_maps = [
        {
            "x1": x1[c * B_SHARD : (c + 1) * B_SHARD],
            "x2": x2[c * B_SHARD : (c + 1) * B_SHARD],
        }
        for c in range(N_CORES)
    ]

    if _NC_CACHE is None:
        _NC_CACHE = build_kernel(seq_layout=SEQ_LAYOUT, split_tail=True)

    res = run_bass_kernel_spmd(_NC_CACHE, in_maps, core_ids=list(range(N_CORES)))
    if SEQ_LAYOUT:
        # out_core[p, n] holds shard row n*128+p -> transpose to row order
        shards = [
            np.ascontiguousarray(res.results[c]["out"].T).reshape(B_SHARD)
            for c in range(N_CORES)
        ]
    else:
        shards = [res.results[c]["out"] for c in range(N_CORES)]
    return np.concatenate(shards, axis=0)



# revision 7
# speedup vs baseline: 1.0059x; 1.0059x over previous
"""Per-row cosine-similarity loss (0.5 * cos(x1_row, x2_row)) on 8 TRN2 cores.

Pure data parallel: the batch dim (B=16384) is split into 8 shards of 2048
rows; each core computes its shard independently, no communication.

Per-core kernel (shard = [2048, 4096] f32 per tensor):
  - rows are tiled as row = p*16 + n  (p = SBUF partition, n = tile index),
    so each [128, 4096] tile is one ACT/DVE instruction and the final
    per-row result lands in a [128, 16] tile.
  - ACT (scalar engine): Square activation with accum_out -> per-row sum of
    squares (fused square+reduce, one pass per tensor).
  - DVE (vector engine): scalar_tensor_tensor(mult, mult, accum_out) ->
    per-row dot product (fused multiply+reduce, one pass).

The kernel is HBM-bound: 64 MiB input per core @ ~358 GB/s => ~187 us floor,
and measured steady-state is ~94% of that regardless of DMA issue pattern
(single SP HWDGE ring already saturates; split rings / bigger DMAs /
sequential layout all measured neutral-to-worse). What remains tunable is
the single-pass *tail* after the last input byte lands:

  - A dummy Sqrt at kernel start makes the single ACT table load pick the
    `sqrt_and_others` function set (it holds BOTH square and sqrt), so the
    finalize Sqrt does NOT pay a ~1.3 us table reload in the tail.
  - Columns 0..14 are finalized and stored while the last tile is still
    streaming (store on the idle GPSIMD/SWDGE ring: the SP ring would
    head-of-line-block the input stream, the ACT ring would steal ~0.5 us
    of ACT sequencer time in its busiest stretch).
  - x1's last tile is loaded whole, one tile early, so its full-tile square
    completes mid-stream and sx[:,15] needs no tail work at all.
  - x2's last tile streams in pieces sized so ACT (square) and DVE (dot)
    each keep pace with the piece arrivals; partial sums fold into the
    running sy/dot during streaming. Only a 256-col piece remains after
    the final byte: one small square (ACT) one small dot (DVE) in parallel.
  - The remaining [128,1] finalize chain is minimal: the last partial-sum
    folds are fused into scalar_tensor_tensor (add+mul in one op), and the
    chain is den=(sy+p)*sx -> sqrt(4*den) -> reciprocal -> res=(dot+p)*rec
    (the 0.5 factor is folded via sqrt(4*sx*sy) = 2*sqrt(sx*sy)).
"""

import numpy as np

import concourse.bacc as bacc
import concourse.bass as bass
import concourse.tile as tile
from concourse import mybir
from concourse.bass_utils import run_bass_kernel_spmd

B, D = 16384, 4096
N_CORES = 8
B_SHARD = B // N_CORES  # 2048
P = 128
N_TILES = B_SHARD // P  # 16

_NC_CACHE = None

# x2 last-tile piece widths (cols); must sum to D. Sized so per-piece
# square (ACT) and dot (DVE) each finish within one piece-DMA window,
# with a small final piece to minimize post-last-byte compute.
PIECES = (1024, 1024, 1024, 768, 256)


def build_kernel(
    repeat: int = 1,
    bufs: int = 4,
    pieces: tuple = PIECES,
    legacy_tail: bool = False,
    preload_sqrt: bool = True,
    early_store_engine: str = "gpsimd",
) -> bass.Bass:
    # Bacc (not plain Bass): its compile() pass legalizes instructions that
    # carry multiple sync waits, which walrus rejects from raw Bass output.
    # `repeat` re-runs the streaming loop (same data, same output) for
    # slope-method benchmarks; keep 1 for real use.
    nc = bacc.Bacc("TRN2", target_bir_lowering=False)
    f32 = mybir.dt.float32
    SQRT = mybir.ActivationFunctionType.Sqrt
    SQUARE = mybir.ActivationFunctionType.Square
    MULT = mybir.AluOpType.mult
    ADD = mybir.AluOpType.add

    x1 = nc.dram_tensor("x1", [B_SHARD, D], f32, kind="ExternalInput")
    x2 = nc.dram_tensor("x2", [B_SHARD, D], f32, kind="ExternalInput")
    out = nc.dram_tensor("out", [B_SHARD], f32, kind="ExternalOutput")

    # row = p*N_TILES + n: tile n is [128, D] with 16 KiB contiguous per
    # partition at 256 KiB partition stride.
    x1r = x1.rearrange("(p n) d -> p n d", p=P)  # [128, 16, D]
    x2r = x2.rearrange("(p n) d -> p n d", p=P)
    outr = out.rearrange("(p n) -> p n", p=P)  # [128, 16]

    assert sum(pieces) == D
    n_pieces = len(pieces)

    with tile.TileContext(nc) as tc:
        with (
            tc.tile_pool(name="x1p", bufs=bufs) as x1p,
            tc.tile_pool(name="x2p", bufs=bufs) as x2p,
            tc.tile_pool(name="junk", bufs=1) as junkp,
            tc.tile_pool(name="stats", bufs=1) as statsp,
        ):
            sx = statsp.tile([P, N_TILES], f32)
            sy = statsp.tile([P, N_TILES], f32)
            dot = statsp.tile([P, N_TILES], f32)
            # Mandatory full-size outputs of the fused reduce ops; never read.
            junk_a = junkp.tile([P, D], f32)
            junk_v = junkp.tile([P, D], f32)
            # partial accums for x2 pieces 1.. of the last tile (sy, dot)
            part = statsp.tile([P, 2 * max(1, n_pieces - 1)], f32, name="part")

            den = statsp.tile([P, N_TILES], f32, name="den")
            ssd = statsp.tile([P, N_TILES], f32, name="ssd")
            rec = statsp.tile([P, N_TILES], f32, name="rec")
            res = statsp.tile([P, N_TILES], f32, name="res")
            pre = statsp.tile([P, 2], f32, name="pre")
            scale4 = statsp.tile([P, 1], f32, name="scale4")
            bias4 = statsp.tile([P, 1], f32, name="bias4")
            # Dedicated tiles for the last tile's streams. Each x2 piece gets
            # its OWN tile: pieces sharing one tile made every consumer wait
            # on the latest-issued DMA write to that tile (conservative dep
            # on the shared sem lane), running the whole piece pipeline one
            # piece late and stacking ACT work past the final byte.
            t1_last = statsp.tile([P, D], f32, name="t1_last")
            t2_pieces = [
                statsp.tile([P, L], f32, name=f"t2_piece{j}")
                for j, L in enumerate(pieces)
            ]

            if preload_sqrt:
                # Force the single ACT table load to pick the set that holds
                # BOTH square and sqrt, so the finalize sqrt needs no reload.
                nc.vector.memset(pre[:, 0:1], 1.0)
                nc.scalar.activation(out=pre[:, 1:2], in_=pre[:, 0:1], func=SQRT)

            def full_tile(n):
                t1 = x1p.tile([P, D], f32, name="t1")
                t2 = x2p.tile([P, D], f32, name="t2")
                nc.sync.dma_start(out=t1, in_=x1r[:, n, :])
                nc.sync.dma_start(out=t2, in_=x2r[:, n, :])
                nc.scalar.activation(
                    out=junk_a, in_=t1, func=SQUARE, accum_out=sx[:, n : n + 1]
                )
                nc.scalar.activation(
                    out=junk_a, in_=t2, func=SQUARE, accum_out=sy[:, n : n + 1]
                )
                # Fused (t1*1.0)*t2 with accum_out = per-row sum -> dot.
                # (tensor_tensor_reduce compiles but faults on HW; this
                # TensorScalarPtr form is the supported fused mul+reduce.)
                nc.vector.scalar_tensor_tensor(
                    out=junk_v, in0=t1, scalar=1.0, in1=t2,
                    op0=MULT, op1=MULT, accum_out=dot[:, n : n + 1],
                )

            def last_tile(with_finalize):
                n = N_TILES - 1
                c15 = slice(n, n + 1)
                t1 = t1_last
                # x1's last tile loads whole; its square completes mid-stream
                # and t1 stays resident for the piece dots.
                nc.sync.dma_start(out=t1, in_=x1r[:, n, :])
                nc.scalar.activation(
                    out=junk_a, in_=t1, func=SQUARE, accum_out=sx[:, c15]
                )
                off = 0
                for j, L in enumerate(pieces):
                    cs = slice(off, off + L)
                    off += L
                    t2j = t2_pieces[j]
                    nc.sync.dma_start(out=t2j, in_=x2r[:, n, cs])
                    if j == 0:
                        a_sy, a_dot = sy[:, c15], dot[:, c15]
                    else:
                        k = 2 * (j - 1)
                        a_sy = part[:, k : k + 1]
                        a_dot = part[:, k + 1 : k + 2]
                    nc.scalar.activation(
                        out=junk_a[:, cs], in_=t2j, func=SQUARE, accum_out=a_sy
                    )
                    nc.vector.scalar_tensor_tensor(
                        out=junk_v[:, cs], in0=t1[:, cs], scalar=1.0, in1=t2j,
                        op0=MULT, op1=MULT, accum_out=a_dot,
                    )
                    if 0 < j < n_pieces - 1:
                        # fold mid-piece partials while later pieces stream
                        k = 2 * (j - 1)
                        nc.vector.tensor_add(sy[:, c15], sy[:, c15], part[:, k : k + 1])
                        nc.vector.tensor_add(
                            dot[:, c15], dot[:, c15], part[:, k + 1 : k + 2]
                        )
                k = 2 * (n_pieces - 2)
                if not with_finalize:
                    # deterministic per-iteration state in repeat mode
                    nc.vector.tensor_add(sy[:, c15], sy[:, c15], part[:, k : k + 1])
                    nc.vector.tensor_add(
                        dot[:, c15], dot[:, c15], part[:, k + 1 : k + 2]
                    )
                    return
                # Tail finalize for column 15. The sy fold and den multiply
                # are fused INTO the sqrt via per-partition scale/bias APs:
                #   sqrt(4*sx*p_sy + 4*sx*sy_partial) = 2*sqrt(sx*sy_total)
                # scale4/bias4 are computed off the critical chain (sx and
                # sy_partial are final well before the last piece lands), and
                # the sqrt reads the last square's accumulator on the SAME
                # engine - no DVE round-trip between them.
                nc.vector.tensor_scalar_mul(scale4, sx[:, c15], 4.0)
                nc.vector.tensor_mul(bias4, scale4, sy[:, c15])
                nc.scalar.activation(
                    out=ssd[:, c15], in_=part[:, k : k + 1], func=SQRT,
                    scale=scale4, bias=bias4,
                )
                nc.vector.reciprocal(rec[:, c15], ssd[:, c15])
                # res = (dot + p_dot) * rec
                nc.vector.scalar_tensor_tensor(
                    out=res[:, c15], in0=dot[:, c15], scalar=part[:, k + 1 : k + 2],
                    in1=rec[:, c15], op0=ADD, op1=MULT,
                )
                nc.sync.dma_start(out=outr[:, c15], in_=res[:, c15])

            def early_finalize():
                # cols 0..14 are final once tile 14's compute lands; finalize
                # and store them while the last tile is still streaming.
                # cos/2 = dot / (2*sqrt(sx*sy));  sqrt(4*d) = 2*sqrt(d)
                c = slice(0, N_TILES - 1)
                nc.vector.tensor_mul(den[:, c], sx[:, c], sy[:, c])
                nc.scalar.activation(out=ssd[:, c], in_=den[:, c], func=SQRT, scale=4.0)
                nc.vector.reciprocal(rec[:, c], ssd[:, c])
                nc.vector.tensor_mul(res[:, c], dot[:, c], rec[:, c])
                eng = getattr(nc, early_store_engine)
                eng.dma_start(out=outr[:, c], in_=res[:, c])

            def batch_finalize():
                c = slice(0, N_TILES)
                nc.vector.tensor_mul(den[:, c], sx[:, c], sy[:, c])
                nc.scalar.activation(out=ssd[:, c], in_=den[:, c], func=SQRT, scale=4.0)
                nc.vector.reciprocal(rec[:, c], ssd[:, c])
                nc.vector.tensor_mul(res[:, c], dot[:, c], rec[:, c])
                nc.sync.dma_start(out=outr[:, c], in_=res[:, c])

            def stream_body(with_finalize):
                for g in range(N_TILES - 1):
                    full_tile(g)
                if legacy_tail:
                    full_tile(N_TILES - 1)
                    if with_finalize:
                        batch_finalize()
                    return
                if with_finalize:
                    early_finalize()
                last_tile(with_finalize)

            if repeat == 1:
                stream_body(with_finalize=True)
            else:
                with tc.For_i(0, repeat, 1):
                    stream_body(with_finalize=False)
                batch_finalize()

    nc.compile()
    return nc


def kernel(x1: np.ndarray, x2: np.ndarray, **_kw) -> np.ndarray:
    global _NC_CACHE
    x1 = np.ascontiguousarray(np.asarray(x1, dtype=np.float32))
    x2 = np.ascontiguousarray(np.asarray(x2, dtype=np.float32))
    assert x1.shape == (B, D) and x2.shape == (B, D)

    in_maps = [
        {
            "x1": x1[c * B_SHARD : (c + 1) * B_SHARD],
            "x2": x2[c * B_SHARD : (c + 1) * B_SHARD],
        }
        for c in range(N_CORES)
    ]

    if _NC_CACHE is None:
        _NC_CACHE = build_kernel()

    res = run_bass_kernel_spmd(_NC_CACHE, in_maps, core_ids=list(range(N_CORES)))
    shards = [res.results[c]["out"] for c in range(N_CORES)]
    return np.concatenate(shards, axis=0)


# revision 12
# speedup vs baseline: 1.0064x; 1.0005x over previous
"""Per-row cosine-similarity loss (0.5 * cos(x1_row, x2_row)) on 8 TRN2 cores.

Pure data parallel: the batch dim (B=16384) is split into 8 shards of 2048
rows; each core computes its shard independently, no communication.

Per-core kernel (shard = [2048, 4096] f32 per tensor):
  - rows are tiled as row = p*16 + n  (p = SBUF partition, n = tile index),
    so each [128, 4096] tile is one ACT/DVE instruction and the final
    per-row result lands in a [128, 16] tile.
  - ACT (scalar engine): Square activation with accum_out -> per-row sum of
    squares (fused square+reduce, one pass per tensor).
  - DVE (vector engine): scalar_tensor_tensor(mult, mult, accum_out) ->
    per-row dot product (fused multiply+reduce, one pass).

The kernel is HBM-bound: 64 MiB input per core @ ~358 GB/s => ~187 us floor,
and measured steady-state is ~94% of that regardless of DMA issue pattern
(single SP HWDGE ring already saturates; split rings / bigger DMAs /
sequential layout all measured neutral-to-worse). What remains tunable is
the single-pass *tail* after the last input byte lands:

  - A dummy Sqrt at kernel start makes the single ACT table load pick the
    `sqrt_and_others` function set (it holds BOTH square and sqrt), so the
    finalize Sqrt does NOT pay a ~1.3 us table reload in the tail.
  - Columns 0..14 are finalized and stored while the last tile is still
    streaming (store on the idle GPSIMD/SWDGE ring: the SP ring would
    head-of-line-block the input stream, the ACT ring would steal ~0.5 us
    of ACT sequencer time in its busiest stretch).
  - x1's last tile is loaded whole, one tile early, so its full-tile square
    completes mid-stream and sx[:,15] needs no tail work at all.
  - x2's last tile streams in pieces sized so ACT (square) and DVE (dot)
    each keep pace with the piece arrivals; partial sums fold into the
    running sy/dot during streaming. Only a 256-col piece remains after
    the final byte: one small square (ACT) one small dot (DVE) in parallel.
  - The remaining [128,1] finalize chain is minimal: the last partial-sum
    folds are fused into scalar_tensor_tensor (add+mul in one op), and the
    chain is den=(sy+p)*sx -> sqrt(4*den) -> reciprocal -> res=(dot+p)*rec
    (the 0.5 factor is folded via sqrt(4*sx*sy) = 2*sqrt(sx*sy)).
"""

import numpy as np

import concourse.bacc as bacc
import concourse.bass as bass
import concourse.tile as tile
from concourse import mybir
from concourse.bass_utils import run_bass_kernel_spmd

B, D = 16384, 4096
N_CORES = 8
B_SHARD = B // N_CORES  # 2048
P = 128
N_TILES = B_SHARD // P  # 16

_NC_CACHE = None

# x2 last-tile piece widths (cols); must sum to D. Sized so per-piece
# square (ACT) and dot (DVE) each finish within one piece-DMA window,
# with a small final piece to minimize post-last-byte compute.
PIECES = (1024, 1024, 1024, 768, 256)


def build_kernel(
    repeat: int = 1,
    bufs: int = 4,
    pieces: tuple = PIECES,
    legacy_tail: bool = False,
    preload_sqrt: bool = True,
    early_store_engine: str = "gpsimd",
    tail_serialize: bool = False,
) -> bass.Bass:
    # Bacc (not plain Bass): its compile() pass legalizes instructions that
    # carry multiple sync waits, which walrus rejects from raw Bass output.
    # `repeat` re-runs the streaming loop (same data, same output) for
    # slope-method benchmarks; keep 1 for real use.
    nc = bacc.Bacc("TRN2", target_bir_lowering=False)
    f32 = mybir.dt.float32
    SQRT = mybir.ActivationFunctionType.Sqrt
    SQUARE = mybir.ActivationFunctionType.Square
    MULT = mybir.AluOpType.mult
    ADD = mybir.AluOpType.add

    x1 = nc.dram_tensor("x1", [B_SHARD, D], f32, kind="ExternalInput")
    x2 = nc.dram_tensor("x2", [B_SHARD, D], f32, kind="ExternalInput")
    out = nc.dram_tensor("out", [B_SHARD], f32, kind="ExternalOutput")

    # row = p*N_TILES + n: tile n is [128, D] with 16 KiB contiguous per
    # partition at 256 KiB partition stride.
    x1r = x1.rearrange("(p n) d -> p n d", p=P)  # [128, 16, D]
    x2r = x2.rearrange("(p n) d -> p n d", p=P)
    outr = out.rearrange("(p n) -> p n", p=P)  # [128, 16]

    assert sum(pieces) == D
    n_pieces = len(pieces)

    with tile.TileContext(nc) as tc:
        with (
            tc.tile_pool(name="x1p", bufs=bufs) as x1p,
            tc.tile_pool(name="x2p", bufs=bufs) as x2p,
            tc.tile_pool(name="junk", bufs=1) as junkp,
            tc.tile_pool(name="stats", bufs=1) as statsp,
        ):
            sx = statsp.tile([P, N_TILES], f32)
            sy = statsp.tile([P, N_TILES], f32)
            dot = statsp.tile([P, N_TILES], f32)
            # Mandatory full-size outputs of the fused reduce ops; never read.
            junk_a = junkp.tile([P, D], f32)
            junk_v = junkp.tile([P, D], f32)
            # partial accums for x2 pieces 1.. of the last tile (sy, dot)
            part = statsp.tile([P, 2 * max(1, n_pieces - 1)], f32, name="part")

            den = statsp.tile([P, N_TILES], f32, name="den")
            ssd = statsp.tile([P, N_TILES], f32, name="ssd")
            rec = statsp.tile([P, N_TILES], f32, name="rec")
            res = statsp.tile([P, N_TILES], f32, name="res")
            pre = statsp.tile([P, 2], f32, name="pre")
            scale4 = statsp.tile([P, 1], f32, name="scale4")
            bias4 = statsp.tile([P, 1], f32, name="bias4")
            # Dedicated tiles for the last tile's streams. Each x2 piece gets
            # its OWN tile: pieces sharing one tile made every consumer wait
            # on the latest-issued DMA write to that tile (conservative dep
            # on the shared sem lane), running the whole piece pipeline one
            # piece late and stacking ACT work past the final byte.
            t1_last = statsp.tile([P, D], f32, name="t1_last")
            t2_pieces = [
                statsp.tile([P, L], f32, name=f"t2_piece{j}")
                for j, L in enumerate(pieces)
            ]

            if preload_sqrt:
                # Force the single ACT table load to pick the set that holds
                # BOTH square and sqrt, so the finalize sqrt needs no reload.
                nc.vector.memset(pre[:, 0:1], 1.0)
                nc.scalar.activation(out=pre[:, 1:2], in_=pre[:, 0:1], func=SQRT)

            def full_tile(n):
                t1 = x1p.tile([P, D], f32, name="t1")
                t2 = x2p.tile([P, D], f32, name="t2")
                nc.sync.dma_start(out=t1, in_=x1r[:, n, :])
                nc.sync.dma_start(out=t2, in_=x2r[:, n, :])
                nc.scalar.activation(
                    out=junk_a, in_=t1, func=SQUARE, accum_out=sx[:, n : n + 1]
                )
                nc.scalar.activation(
                    out=junk_a, in_=t2, func=SQUARE, accum_out=sy[:, n : n + 1]
                )
                # Fused (t1*1.0)*t2 with accum_out = per-row sum -> dot.
                # (tensor_tensor_reduce compiles but faults on HW; this
                # TensorScalarPtr form is the supported fused mul+reduce.)
                nc.vector.scalar_tensor_tensor(
                    out=junk_v, in0=t1, scalar=1.0, in1=t2,
                    op0=MULT, op1=MULT, accum_out=dot[:, n : n + 1],
                )

            def last_tile(with_finalize):
                n = N_TILES - 1
                c15 = slice(n, n + 1)
                t1 = t1_last
                # x1's last tile loads whole; its square completes mid-stream
                # and t1 stays resident for the piece dots.
                nc.sync.dma_start(out=t1, in_=x1r[:, n, :])
                nc.scalar.activation(
                    out=junk_a, in_=t1, func=SQUARE, accum_out=sx[:, c15]
                )
                if with_finalize:
                    nc.vector.tensor_scalar_mul(scale4, sx[:, c15], 4.0)
                off = 0
                for j, L in enumerate(pieces):
                    cs = slice(off, off + L)
                    off += L
                    t2j = t2_pieces[j]
                    nc.sync.dma_start(out=t2j, in_=x2r[:, n, cs])
                    if j == 0:
                        a_sy, a_dot = sy[:, c15], dot[:, c15]
                    else:
                        k = 2 * (j - 1)
                        a_sy = part[:, k : k + 1]
                        a_dot = part[:, k + 1 : k + 2]
                    if with_finalize and j == n_pieces - 2:
                        # sy folds through piece n-3 are final here; bias4
                        # covers them so the tail needs only p_sy of the last
                        # two pieces (summed into tsum below).
                        nc.vector.tensor_mul(bias4, scale4, sy[:, c15])
                    nc.vector.scalar_tensor_tensor(
                        out=junk_v[:, cs], in0=t1[:, cs], scalar=1.0, in1=t2j,
                        op0=MULT, op1=MULT, accum_out=a_dot,
                    )
                    if j == n_pieces - 1:
                        # Final piece's square on DVE (STT t2*t2): DVE runs
                        # dot+square back-to-back and finishes before ACT's
                        # queue (still draining earlier pieces) could start it.
                        nc.vector.scalar_tensor_tensor(
                            out=junk_a[:, cs], in0=t2j, scalar=1.0, in1=t2j,
                            op0=MULT, op1=MULT, accum_out=a_sy,
                        )
                    else:
                        nc.scalar.activation(
                            out=junk_a[:, cs], in_=t2j, func=SQUARE, accum_out=a_sy
                        )
                    if 0 < j < n_pieces - 1:
                        # fold mid-piece partials while later pieces stream;
                        # in finalize mode piece n-2's sy part stays unfolded
                        # (tsum handles it) so the tail doesn't wait on it.
                        k = 2 * (j - 1)
                        if not (with_finalize and j == n_pieces - 2):
                            nc.vector.tensor_add(
                                sy[:, c15], sy[:, c15], part[:, k : k + 1]
                            )
                        nc.vector.tensor_add(
                            dot[:, c15], dot[:, c15], part[:, k + 1 : k + 2]
                        )
                k = 2 * (n_pieces - 2)
                kp = 2 * (n_pieces - 3)
                if not with_finalize:
                    # deterministic per-iteration state in repeat mode
                    nc.vector.tensor_add(sy[:, c15], sy[:, c15], part[:, k : k + 1])
                    nc.vector.tensor_add(
                        dot[:, c15], dot[:, c15], part[:, k + 1 : k + 2]
                    )
                    return
                # Tail finalize for column 15. The last two sy partials sum
                # into tsum; the rest of sy and the den multiply are fused
                # INTO the sqrt via per-partition scale/bias APs
                # (precomputed off the critical chain above):
                #   sqrt(4*sx*tsum + 4*sx*sy_partial) = 2*sqrt(sx*sy_total)
                tsum = pre[:, 0:1]
                nc.vector.tensor_add(tsum, part[:, kp : kp + 1], part[:, k : k + 1])
                nc.scalar.activation(
                    out=ssd[:, c15], in_=tsum, func=SQRT,
                    scale=scale4, bias=bias4,
                )
                nc.vector.reciprocal(rec[:, c15], ssd[:, c15])
                # res = (dot + p_dot) * rec
                nc.vector.scalar_tensor_tensor(
                    out=res[:, c15], in0=dot[:, c15], scalar=part[:, k + 1 : k + 2],
                    in1=rec[:, c15], op0=ADD, op1=MULT,
                )
                nc.sync.dma_start(out=outr[:, c15], in_=res[:, c15])

            def early_finalize():
                # cols 0..14 are final once tile 14's compute lands; finalize
                # and store them while the last tile is still streaming.
                # cos/2 = dot / (2*sqrt(sx*sy));  sqrt(4*d) = 2*sqrt(d)
                c = slice(0, N_TILES - 1)
                nc.vector.tensor_mul(den[:, c], sx[:, c], sy[:, c])
                nc.scalar.activation(out=ssd[:, c], in_=den[:, c], func=SQRT, scale=4.0)
                nc.vector.reciprocal(rec[:, c], ssd[:, c])
                nc.vector.tensor_mul(res[:, c], dot[:, c], rec[:, c])
                eng = getattr(nc, early_store_engine)
                eng.dma_start(out=outr[:, c], in_=res[:, c])

            def batch_finalize():
                c = slice(0, N_TILES)
                nc.vector.tensor_mul(den[:, c], sx[:, c], sy[:, c])
                nc.scalar.activation(out=ssd[:, c], in_=den[:, c], func=SQRT, scale=4.0)
                nc.vector.reciprocal(rec[:, c], ssd[:, c])
                nc.vector.tensor_mul(res[:, c], dot[:, c], rec[:, c])
                nc.sync.dma_start(out=outr[:, c], in_=res[:, c])

            def stream_body(with_finalize):
                for g in range(N_TILES - 1):
                    full_tile(g)
                if legacy_tail:
                    full_tile(N_TILES - 1)
                    if with_finalize:
                        batch_finalize()
                    return
                if with_finalize:
                    early_finalize()
                last_tile(with_finalize)

            if repeat == 1:
                stream_body(with_finalize=True)
            elif tail_serialize:
                # Benchmark mode: run the FULL pass (finalize + stores)
                # every iteration, and make each iteration's first input DMA
                # wait (ring head-of-line) on a dummy DMA that write-after-
                # read depends on the previous iteration's final store. The
                # per-iteration slope then includes the whole tail.
                with tc.For_i(0, repeat, 1):
                    nc.sync.dma_start(
                        out=res[:, N_TILES - 1 : N_TILES], in_=x1r[:, 0, 0:1]
                    )
                    stream_body(with_finalize=True)
            else:
                with tc.For_i(0, repeat, 1):
                    stream_body(with_finalize=False)
                batch_finalize()

    nc.compile()
    return nc


def kernel(x1: np.ndarray, x2: np.ndarray, **_kw) -> np.ndarray:
    global _NC_CACHE
    x1 = np.ascontiguousarray(np.asarray(x1, dtype=np.float32))
    x2 = np.ascontiguousarray(np.asarray(x2, dtype=np.float32))
    assert x1.shape == (B, D) and x2.shape == (B, D)

    in_maps = [
        {
            "x1": x1[c * B_SHARD : (c + 1) * B_SHARD],
            "x2": x2[c * B_SHARD : (c + 1) * B_SHARD],
        }
        for c in range(N_CORES)
    ]

    if _NC_CACHE is None:
        _NC_CACHE = build_kernel()

    res = run_bass_kernel_spmd(_NC_CACHE, in_maps, core_ids=list(range(N_CORES)))
    shards = [res.results[c]["out"] for c in range(N_CORES)]
    return np.concatenate(shards, axis=0)
